# revision 1
# baseline (speedup 1.0000x reference)
"""Mean-aggregator (GNN message passing) Bass kernel for 8 trn2 NeuronCores.

Algorithm: out[s] = mean over edges e with seg_ids[e]==s of features[neigh_idx[e]].

Sharding: data-parallel over destination segments. Core c owns segments
[c*5120, (c+1)*5120) = 40 aligned blocks of 128 segments. Since seg_ids is
sorted, each core's edges are a contiguous slice; each 128-segment block's
edges are padded host-side to K*128 slots (K = max block load over all
cores/blocks) so all 8 cores execute one identical SPMD program.

Per 128-edge tile t (block b = t//K):
  - indirect DMA gathers 128 rows of the (ones-augmented) feature table
    into SBUF tile X [128 edges, 132] (col 128 = 1.0 -> counts).
  - DVE builds selection matrix S[e, s] = (relseg[e] == s) by comparing a
    constant iota row against the per-edge relative segment id; pad edges
    carry relseg = -1 so their S row is all zero (contributing nothing to
    sums or counts).
  - PE matmul accumulates S.T @ X into PSUM [128 segs, 132] across the
    block's K tiles.
Per block flush: counts = psum[:, 128], clamp to >=1, reciprocal, scale,
DMA the [128, 128] result block to DRAM.
"""

import numpy as np

NUM_NODES = 50000
FEAT = 128
NUM_BATCH = 40000
N_CORES = 8
BLOCKS_PER_CORE = 40
SEG_BLOCK = 128
SEGS_PER_CORE = BLOCKS_PER_CORE * SEG_BLOCK  # 5120
AUG = 132  # feature row padded with [1.0, 0, 0, 0] -> row stride 528B

_program_cache: dict = {}


def _build_program(K: int):
    """Build (and cache) the SPMD Bass program for K tiles per block."""
    if K in _program_cache:
        return _program_cache[K]

    import concourse.bacc as bacc
    import concourse.bass as bass
    import concourse.mybir as mybir
    import concourse.tile as tile

    T = BLOCKS_PER_CORE * K
    f32 = mybir.dt.float32
    i32 = mybir.dt.int32

    nc = bacc.Bacc("TRN2", target_bir_lowering=False, debug=False)
    feat = nc.dram_tensor("features", [NUM_NODES, AUG], f32, kind="ExternalInput")
    nidx = nc.dram_tensor("nidx", [128, T], i32, kind="ExternalInput")
    relseg = nc.dram_tensor("relseg", [128, T], f32, kind="ExternalInput")
    out = nc.dram_tensor("out", [SEGS_PER_CORE, FEAT], f32, kind="ExternalOutput")

    with tile.TileContext(nc) as tc:
        with (
            tc.tile_pool(name="const", bufs=1) as constp,
            tc.tile_pool(name="idx", bufs=1) as idxp,
            tc.tile_pool(name="x", bufs=8) as xp,
            tc.tile_pool(name="s", bufs=8) as sp,
            tc.tile_pool(name="fl", bufs=4) as flp,
            tc.tile_pool(name="ps", bufs=4, space="PSUM") as pp,
        ):
            nidx_sb = idxp.tile([128, T], i32)
            relseg_sb = idxp.tile([128, T], f32)
            nc.sync.dma_start(nidx_sb[:], nidx[:])
            nc.sync.dma_start(relseg_sb[:], relseg[:])

            iota_i = constp.tile([128, 128], i32)
            iota_f = constp.tile([128, 128], f32)
            nc.gpsimd.iota(iota_i[:], pattern=[[1, 128]], base=0, channel_multiplier=0)
            nc.vector.tensor_copy(iota_f[:], iota_i[:])

            for b in range(BLOCKS_PER_CORE):
                ps = pp.tile([128, AUG], f32, space="PSUM")
                for k in range(K):
                    t = b * K + k
                    xt = xp.tile([128, AUG], f32, tag="xt")
                    nc.gpsimd.indirect_dma_start(
                        out=xt[:],
                        out_offset=None,
                        in_=feat[:],
                        in_offset=bass.IndirectOffsetOnAxis(
                            ap=nidx_sb[:, t : t + 1], axis=0
                        ),
                    )
                    st = sp.tile([128, 128], f32, tag="st")
                    nc.vector.tensor_scalar(
                        out=st[:],
                        in0=iota_f[:],
                        scalar1=relseg_sb[:, t : t + 1],
                        scalar2=None,
                        op0=mybir.AluOpType.is_equal,
                    )
                    nc.tensor.matmul(
                        ps[:],
                        lhsT=st[:],
                        rhs=xt[:],
                        start=(k == 0),
                        stop=(k == K - 1),
                    )
                cnt = flp.tile([128, 1], f32, tag="cnt")
                rcnt = flp.tile([128, 1], f32, tag="rcnt")
                ob = flp.tile([128, FEAT], f32, tag="ob")
                nc.vector.tensor_scalar_max(cnt[:], ps[:, 128:129], 1.0)
                nc.vector.reciprocal(rcnt[:], cnt[:])
                nc.vector.tensor_scalar_mul(ob[:], ps[:, 0:FEAT], rcnt[:])
                nc.sync.dma_start(out[b * 128 : (b + 1) * 128, :], ob[:])

    nc.compile()
    _program_cache[K] = nc
    return nc


def _prepare_inputs(features, neigh_idx, seg_ids):
    """Host-side index preprocessing: shard edges by segment block and pad
    each block to a common K*128 slots. Returns (features_aug, per-core
    nidx [128,T] int32, per-core relseg [128,T] f32, K)."""
    E = seg_ids.shape[0]
    n_blocks = N_CORES * BLOCKS_PER_CORE
    bases = np.arange(n_blocks + 1, dtype=np.int64) * SEG_BLOCK
    bnd = np.searchsorted(seg_ids, bases)  # edges of block i: bnd[i]:bnd[i+1]
    sizes = np.diff(bnd)
    K = max(1, int(-(-sizes.max() // 128)))
    T = BLOCKS_PER_CORE * K
    slots = T * 128

    nidx_all = np.zeros((N_CORES, slots), dtype=np.int32)
    relseg_all = np.full((N_CORES, slots), -1.0, dtype=np.float32)
    nidx64 = np.ascontiguousarray(neigh_idx)
    seg64 = np.ascontiguousarray(seg_ids)
    for i in range(n_blocks):
        c, b = divmod(i, BLOCKS_PER_CORE)
        lo, hi = bnd[i], bnd[i + 1]
        n = hi - lo
        off = b * K * 128
        nidx_all[c, off : off + n] = nidx64[lo:hi]
        relseg_all[c, off : off + n] = (seg64[lo:hi] - bases[i]).astype(np.float32)

    # [slots] -> [T, 128] -> transpose -> [128, T] so tile t's 128 edges sit
    # one per partition in column t.
    nidx_t = [np.ascontiguousarray(a.reshape(T, 128).T) for a in nidx_all]
    relseg_t = [np.ascontiguousarray(a.reshape(T, 128).T) for a in relseg_all]

    feat_aug = np.zeros((NUM_NODES, AUG), dtype=np.float32)
    feat_aug[:, :FEAT] = features
    feat_aug[:, FEAT] = 1.0
    return feat_aug, nidx_t, relseg_t, K


LAST_RESULT = None


def kernel(features, neigh_idx, seg_ids, num_batch, _trace=False):
    global LAST_RESULT
    from concourse.bass_utils import run_bass_kernel_spmd

    features = np.asarray(features, dtype=np.float32)
    neigh_idx = np.asarray(neigh_idx)
    seg_ids = np.asarray(seg_ids)
    nb = int(num_batch)
    assert nb == NUM_BATCH, nb
    assert features.shape == (NUM_NODES, FEAT), features.shape

    feat_aug, nidx_t, relseg_t, K = _prepare_inputs(features, neigh_idx, seg_ids)
    nc = _build_program(K)

    in_maps = [
        {"features": feat_aug, "nidx": nidx_t[c], "relseg": relseg_t[c]}
        for c in range(N_CORES)
    ]
    res = run_bass_kernel_spmd(
        nc, in_maps, core_ids=list(range(N_CORES)), trace=_trace
    )
    LAST_RESULT = res

    out = np.empty((NUM_BATCH, FEAT), dtype=np.float32)
    for c in range(N_CORES):
        lo = c * SEGS_PER_CORE
        hi = min(lo + SEGS_PER_CORE, NUM_BATCH)
        if hi > lo:
            out[lo:hi] = res.results[c]["out"][: hi - lo]
    return out
